# revision 12
# baseline (speedup 1.0000x reference)
"""BinaryDense Trainium2 kernel: out = nmk * (inputs @ binarize(weight).T + bias).

binarize(w) = tanh(w * kk) when kk < 1e6 else sign(w).

Strategy (column-parallel over 8 NeuronCores, per the tensor-parallel hint):
  - Each core owns a 2048-row slice of weight/bias (out_channels).
  - On device, the weight slice is streamed once (fp32), binarized with the
    scalar engine, and kept resident in SBUF as fp16 in 4 panels of 512 oc.
  - Inputs are transposed/cast to fp16 on the host (layout prep only) and
    streamed in 512-token chunks, once per panel (4x total).
  - Matmuls: stationary fp16 weight tile [k=128, oc=128], moving fp16 input
    tile [k=128, tok=512], fp32 PSUM accumulation over 32 k-tiles.
  - PSUM eviction fuses nmk*(acc + bias) in one DVE tensor_scalar op.
  - Per-core output is [oc, tok]; the host concatenates/transposes.
"""

import numpy as np

import concourse.bass as bass
import concourse.mybir as mybir
import concourse.tile as tile
from concourse.bass_utils import run_bass_kernel_spmd
from concourse.mybir import ActivationFunctionType, AluOpType

N_CORES = 8
P = 128
IN_CH = 4096
OUT_CH = 16384
TOKENS = 8192
KK_THRESHOLD = 1e6

KT = IN_CH // P          # 32 k-tiles of 128
OC_SH = OUT_CH // N_CORES  # 2048 out-channels per core
CHUNK = 512              # tokens per streamed input chunk
NCH = TOKENS // CHUNK    # 16 chunks
PANEL = 512              # out-channels per resident fp16 weight panel
NQ = OC_SH // PANEL      # 4 panels
OPT = PANEL // P         # 4 oc-tiles per panel
NOCT = OC_SH // P        # 16 oc-tiles per core


def _split_multi_waits(nc, cap=1):
    """Split instructions carrying more than `cap` sync waits.

    The walrus build in this environment supports a single sync-wait command
    per TPB instruction, but Tile's kernel-tail drain/barrier can accumulate
    several residual waits. Moving the excess onto preceding NoOps on the
    same engine is equivalent: the sequencer blocks on each wait in order.
    """
    for f in nc.m.functions:
        for bb in f.blocks:
            out = []
            for inst in bb.instructions:
                si = inst.sync_info
                waits = list(si.on_wait) if si is not None and si.on_wait else []
                if len(waits) > cap:
                    spill, keep = waits[:-cap], waits[-cap:]
                    for i in range(0, len(spill), cap):
                        noop = mybir.InstNoOp(
                            name=nc.get_next_instruction_name(),
                            ins=[],
                            outs=[],
                            engine=inst.engine,
                        )
                        noop.sync_info = mybir.SyncInfo(
                            on_wait=spill[i : i + cap], on_update=[]
                        )
                        nc.register_instruction(noop)
                        out.append(noop)
                    inst.sync_info = mybir.SyncInfo(
                        on_wait=keep,
                        on_update=list(si.on_update) if si.on_update else [],
                    )
                out.append(inst)
            bb.instructions = out


def _build(tanh_branch: bool):
    f32, f16 = mybir.dt.float32, mybir.dt.float16
    nc = bass.Bass("TRN2", target_bir_lowering=False, debug=False)
    # w6[q, ot, p, t*128+j] = weightT[t*128+p, q*PANEL + ot*128 + j]:
    # one oc-tile's whole K panel is contiguous per partition -> one DMA.
    w6 = nc.dram_tensor(
        "w6", [NQ, OPT, P, KT * P], f32, kind="ExternalInput"
    ).ap()
    x4 = nc.dram_tensor("x4", [NCH, P, KT, CHUNK], f16, kind="ExternalInput").ap()
    bias_pt = nc.dram_tensor("bias_pt", [P, NOCT], f32, kind="ExternalInput").ap()
    nmk = nc.dram_tensor("nmk", [1], f32, kind="ExternalInput").ap()
    kk = nc.dram_tensor("kk", [1], f32, kind="ExternalInput").ap()
    o4 = nc.dram_tensor("o4", [NOCT, P, TOKENS], f32, kind="ExternalOutput").ap()

    with tile.TileContext(nc) as tc:
        with (
            tc.tile_pool(name="const", bufs=1) as constp,
            tc.tile_pool(name="wq", bufs=2 * OPT) as wqp,
            tc.tile_pool(name="w32", bufs=2) as w32p,
            tc.tile_pool(name="xc", bufs=2) as xcp,
            tc.tile_pool(name="stage", bufs=4) as stp,
            tc.tile_pool(name="psum", bufs=8, space="PSUM") as psp,
        ):
            kk_b = constp.tile([P, 1], f32)
            nmk_b = constp.tile([P, 1], f32)
            nc.gpsimd.dma_start(out=kk_b[:], in_=kk.to_broadcast((P, 1)))
            nc.gpsimd.dma_start(out=nmk_b[:], in_=nmk.to_broadcast((P, 1)))
            bias_sb = constp.tile([P, NOCT], f32)
            nc.gpsimd.dma_start(out=bias_sb[:], in_=bias_pt[:])
            nb = constp.tile([P, NOCT], f32)  # nmk * bias, per oc-tile column
            nc.vector.tensor_scalar_mul(nb[:], bias_sb[:], nmk_b[:])

            for q in range(NQ):
                # One fp16 sub-panel tile per oc-tile: a single contiguous
                # DMA + a single big tanh each, so the first matmul group
                # only waits for the first 4.2MB sub-panel.
                wq = []
                for ot in range(OPT):
                    wsub = wqp.tile([P, KT * P], f16, tag="wsub")
                    w32 = w32p.tile([P, KT * P], f32)
                    # Split the 4.2MB load across two engine queue sets so
                    # the first panel lands in ~half the time.
                    half = KT * P // 2
                    nc.scalar.dma_start(out=w32[:, :half], in_=w6[q, ot, :, :half])
                    nc.gpsimd.dma_start(out=w32[:, half:], in_=w6[q, ot, :, half:])
                    if tanh_branch:
                        nc.scalar.activation(
                            wsub[:],
                            w32[:],
                            ActivationFunctionType.Tanh,
                            scale=kk_b[:],
                        )
                    else:
                        nc.scalar.activation(
                            wsub[:], w32[:], ActivationFunctionType.Sign
                        )
                    wq.append(wsub)
                for ch in range(NCH):
                    xc = xcp.tile([P, KT, CHUNK], f16)
                    nc.sync.dma_start(out=xc[:], in_=x4[ch])
                    for ot in range(OPT):
                        ps = psp.tile([P, CHUNK], f32)
                        for t in range(KT):
                            nc.tensor.matmul(
                                ps[:],
                                wq[ot][:, t * P : (t + 1) * P],
                                xc[:, t, :],
                                start=(t == 0),
                                stop=(t == KT - 1),
                            )
                        og = q * OPT + ot
                        st = stp.tile([P, CHUNK], f32)
                        nc.vector.tensor_scalar(
                            st[:],
                            ps[:],
                            nmk_b[:],
                            nb[:, og : og + 1],
                            op0=AluOpType.mult,
                            op1=AluOpType.add,
                        )
                        nc.gpsimd.dma_start(
                            out=o4[og, :, ch * CHUNK : (ch + 1) * CHUNK], in_=st[:]
                        )

    _split_multi_waits(nc)
    return nc


_PROGRAM_CACHE = {}


def _get_program(tanh_branch: bool):
    if tanh_branch not in _PROGRAM_CACHE:
        _PROGRAM_CACHE[tanh_branch] = _build(tanh_branch)
    return _PROGRAM_CACHE[tanh_branch]


def _prep_inputs(inputs, weight, bias, nmk, kk):
    x = np.asarray(inputs, dtype=np.float32)
    w = np.asarray(weight, dtype=np.float32)
    b = np.asarray(bias, dtype=np.float32)
    nmk = np.asarray(nmk, dtype=np.float32).reshape(1)
    kk = np.asarray(kk, dtype=np.float32).reshape(1)

    # x4[c, p, t, j] = x[c*CHUNK + j, t*P + p], fp16
    xt = np.ascontiguousarray(x.T).astype(np.float16)  # [IN_CH, TOKENS]
    x4 = np.ascontiguousarray(
        xt.reshape(KT, P, NCH, CHUNK).transpose(2, 1, 0, 3)
    )

    in_maps = []
    for c in range(N_CORES):
        wsh = w[c * OC_SH : (c + 1) * OC_SH, :]  # [OC_SH, IN_CH]
        # w6[q, ot, p, t*P+j] = wsh.T[t*P+p, q*PANEL + ot*P + j]
        w6 = np.ascontiguousarray(
            np.ascontiguousarray(wsh.T)
            .reshape(KT, P, NQ, OPT, P)
            .transpose(2, 3, 1, 0, 4)
            .reshape(NQ, OPT, P, KT * P)
        )
        bsh = np.ascontiguousarray(
            b[c * OC_SH : (c + 1) * OC_SH].reshape(NOCT, P).T
        )
        in_maps.append(
            {"w6": w6, "x4": x4, "bias_pt": bsh, "nmk": nmk, "kk": kk}
        )
    return in_maps, kk


def _run(inputs, weight, bias, nmk, kk, trace=False, tmpdir=None):
    in_maps, kk_arr = _prep_inputs(inputs, weight, bias, nmk, kk)
    nc = _get_program(bool(kk_arr[0] < KK_THRESHOLD))
    res = run_bass_kernel_spmd(
        nc, in_maps, core_ids=list(range(N_CORES)), trace=trace, tmpdir=tmpdir
    )
    out = np.empty((TOKENS, OUT_CH), dtype=np.float32)
    for c in range(N_CORES):
        o4 = res.results[c]["o4"]  # [NOCT, P, TOKENS]
        out[:, c * OC_SH : (c + 1) * OC_SH] = o4.reshape(OC_SH, TOKENS).T
    return out, res


def kernel(inputs, weight, bias, nmk, kk):
    out, _ = _run(inputs, weight, bias, nmk, kk, trace=False)
    return out


# revision 13
# speedup vs baseline: 1.0011x; 1.0011x over previous
"""BinaryDense Trainium2 kernel: out = nmk * (inputs @ binarize(weight).T + bias).

binarize(w) = tanh(w * kk) when kk < 1e6 else sign(w).

Strategy (column-parallel over 8 NeuronCores, per the tensor-parallel hint):
  - Each core owns a 2048-row slice of weight/bias (out_channels).
  - On device, the weight slice is streamed once (fp32), binarized with the
    scalar engine, and kept resident in SBUF as fp16 in 4 panels of 512 oc.
  - Inputs are transposed/cast to fp16 on the host (layout prep only) and
    streamed in 512-token chunks, once per panel (4x total).
  - Matmuls: stationary fp16 weight tile [k=128, oc=128], moving fp16 input
    tile [k=128, tok=512], fp32 PSUM accumulation over 32 k-tiles.
  - PSUM eviction fuses nmk*(acc + bias) in one DVE tensor_scalar op.
  - Per-core output is [oc, tok]; the host concatenates/transposes.
"""

import numpy as np

import concourse.bass as bass
import concourse.mybir as mybir
import concourse.tile as tile
from concourse.bass_utils import run_bass_kernel_spmd
from concourse.mybir import ActivationFunctionType, AluOpType

N_CORES = 8
P = 128
IN_CH = 4096
OUT_CH = 16384
TOKENS = 8192
KK_THRESHOLD = 1e6

KT = IN_CH // P          # 32 k-tiles of 128
OC_SH = OUT_CH // N_CORES  # 2048 out-channels per core
CHUNK = 512              # tokens per streamed input chunk
NCH = TOKENS // CHUNK    # 16 chunks
PANEL = 512              # out-channels per resident fp16 weight panel
NQ = OC_SH // PANEL      # 4 panels
OPT = PANEL // P         # 4 oc-tiles per panel
NOCT = OC_SH // P        # 16 oc-tiles per core


def _split_multi_waits(nc, cap=1):
    """Split instructions carrying more than `cap` sync waits.

    The walrus build in this environment supports a single sync-wait command
    per TPB instruction, but Tile's kernel-tail drain/barrier can accumulate
    several residual waits. Moving the excess onto preceding NoOps on the
    same engine is equivalent: the sequencer blocks on each wait in order.
    """
    for f in nc.m.functions:
        for bb in f.blocks:
            out = []
            for inst in bb.instructions:
                si = inst.sync_info
                waits = list(si.on_wait) if si is not None and si.on_wait else []
                if len(waits) > cap:
                    spill, keep = waits[:-cap], waits[-cap:]
                    for i in range(0, len(spill), cap):
                        noop = mybir.InstNoOp(
                            name=nc.get_next_instruction_name(),
                            ins=[],
                            outs=[],
                            engine=inst.engine,
                        )
                        noop.sync_info = mybir.SyncInfo(
                            on_wait=spill[i : i + cap], on_update=[]
                        )
                        nc.register_instruction(noop)
                        out.append(noop)
                    inst.sync_info = mybir.SyncInfo(
                        on_wait=keep,
                        on_update=list(si.on_update) if si.on_update else [],
                    )
                out.append(inst)
            bb.instructions = out


def _build(tanh_branch: bool):
    f32, f16 = mybir.dt.float32, mybir.dt.float16
    nc = bass.Bass("TRN2", target_bir_lowering=False, debug=False)
    # w6[q, ot, p, t*128+j] = weightT[t*128+p, q*PANEL + ot*128 + j]:
    # one oc-tile's whole K panel is contiguous per partition -> one DMA.
    w6 = nc.dram_tensor(
        "w6", [NQ, OPT, P, KT * P], f32, kind="ExternalInput"
    ).ap()
    x4 = nc.dram_tensor("x4", [NCH, P, KT, CHUNK], f16, kind="ExternalInput").ap()
    bias_pt = nc.dram_tensor("bias_pt", [P, NOCT], f32, kind="ExternalInput").ap()
    nmk = nc.dram_tensor("nmk", [1], f32, kind="ExternalInput").ap()
    kk = nc.dram_tensor("kk", [1], f32, kind="ExternalInput").ap()
    o4 = nc.dram_tensor("o4", [NOCT, P, TOKENS], f32, kind="ExternalOutput").ap()

    with tile.TileContext(nc) as tc:
        with (
            tc.tile_pool(name="const", bufs=1) as constp,
            tc.tile_pool(name="wq", bufs=2 * OPT) as wqp,
            tc.tile_pool(name="w32", bufs=2) as w32p,
            tc.tile_pool(name="xc", bufs=2) as xcp,
            tc.tile_pool(name="stage", bufs=4) as stp,
            tc.tile_pool(name="psum", bufs=8, space="PSUM") as psp,
        ):
            kk_b = constp.tile([P, 1], f32)
            nmk_b = constp.tile([P, 1], f32)
            nc.gpsimd.dma_start(out=kk_b[:], in_=kk.to_broadcast((P, 1)))
            nc.gpsimd.dma_start(out=nmk_b[:], in_=nmk.to_broadcast((P, 1)))
            bias_sb = constp.tile([P, NOCT], f32)
            nc.gpsimd.dma_start(out=bias_sb[:], in_=bias_pt[:])
            nb = constp.tile([P, NOCT], f32)  # nmk * bias, per oc-tile column
            nc.vector.tensor_scalar_mul(nb[:], bias_sb[:], nmk_b[:])

            for q in range(NQ):
                # One fp16 sub-panel tile per oc-tile: a single contiguous
                # DMA + a single big tanh each, so the first matmul group
                # only waits for the first 4.2MB sub-panel.
                wq = []
                for ot in range(OPT):
                    wsub = wqp.tile([P, KT * P], f16, tag="wsub")
                    w32 = w32p.tile([P, KT * P], f32)
                    # Split the 4.2MB load across two engine queue sets so
                    # the first panel lands in ~half the time.
                    half = KT * P // 2
                    nc.scalar.dma_start(out=w32[:, :half], in_=w6[q, ot, :, :half])
                    nc.sync.dma_start(out=w32[:, half:], in_=w6[q, ot, :, half:])
                    if tanh_branch:
                        nc.scalar.activation(
                            wsub[:],
                            w32[:],
                            ActivationFunctionType.Tanh,
                            scale=kk_b[:],
                        )
                    else:
                        nc.scalar.activation(
                            wsub[:], w32[:], ActivationFunctionType.Sign
                        )
                    wq.append(wsub)
                for ch in range(NCH):
                    xc = xcp.tile([P, KT, CHUNK], f16)
                    nc.sync.dma_start(out=xc[:], in_=x4[ch])
                    for ot in range(OPT):
                        ps = psp.tile([P, CHUNK], f32)
                        for t in range(KT):
                            nc.tensor.matmul(
                                ps[:],
                                wq[ot][:, t * P : (t + 1) * P],
                                xc[:, t, :],
                                start=(t == 0),
                                stop=(t == KT - 1),
                            )
                        og = q * OPT + ot
                        st = stp.tile([P, CHUNK], f32)
                        nc.vector.tensor_scalar(
                            st[:],
                            ps[:],
                            nmk_b[:],
                            nb[:, og : og + 1],
                            op0=AluOpType.mult,
                            op1=AluOpType.add,
                        )
                        nc.gpsimd.dma_start(
                            out=o4[og, :, ch * CHUNK : (ch + 1) * CHUNK], in_=st[:]
                        )

    _split_multi_waits(nc)
    return nc


_PROGRAM_CACHE = {}


def _get_program(tanh_branch: bool):
    if tanh_branch not in _PROGRAM_CACHE:
        _PROGRAM_CACHE[tanh_branch] = _build(tanh_branch)
    return _PROGRAM_CACHE[tanh_branch]


def _prep_inputs(inputs, weight, bias, nmk, kk):
    x = np.asarray(inputs, dtype=np.float32)
    w = np.asarray(weight, dtype=np.float32)
    b = np.asarray(bias, dtype=np.float32)
    nmk = np.asarray(nmk, dtype=np.float32).reshape(1)
    kk = np.asarray(kk, dtype=np.float32).reshape(1)

    # x4[c, p, t, j] = x[c*CHUNK + j, t*P + p], fp16
    xt = np.ascontiguousarray(x.T).astype(np.float16)  # [IN_CH, TOKENS]
    x4 = np.ascontiguousarray(
        xt.reshape(KT, P, NCH, CHUNK).transpose(2, 1, 0, 3)
    )

    in_maps = []
    for c in range(N_CORES):
        wsh = w[c * OC_SH : (c + 1) * OC_SH, :]  # [OC_SH, IN_CH]
        # w6[q, ot, p, t*P+j] = wsh.T[t*P+p, q*PANEL + ot*P + j]
        w6 = np.ascontiguousarray(
            np.ascontiguousarray(wsh.T)
            .reshape(KT, P, NQ, OPT, P)
            .transpose(2, 3, 1, 0, 4)
            .reshape(NQ, OPT, P, KT * P)
        )
        bsh = np.ascontiguousarray(
            b[c * OC_SH : (c + 1) * OC_SH].reshape(NOCT, P).T
        )
        in_maps.append(
            {"w6": w6, "x4": x4, "bias_pt": bsh, "nmk": nmk, "kk": kk}
        )
    return in_maps, kk


def _run(inputs, weight, bias, nmk, kk, trace=False, tmpdir=None):
    in_maps, kk_arr = _prep_inputs(inputs, weight, bias, nmk, kk)
    nc = _get_program(bool(kk_arr[0] < KK_THRESHOLD))
    res = run_bass_kernel_spmd(
        nc, in_maps, core_ids=list(range(N_CORES)), trace=trace, tmpdir=tmpdir
    )
    out = np.empty((TOKENS, OUT_CH), dtype=np.float32)
    for c in range(N_CORES):
        o4 = res.results[c]["o4"]  # [NOCT, P, TOKENS]
        out[:, c * OC_SH : (c + 1) * OC_SH] = o4.reshape(OC_SH, TOKENS).T
    return out, res


def kernel(inputs, weight, bias, nmk, kk):
    out, _ = _run(inputs, weight, bias, nmk, kk, trace=False)
    return out
